# revision 6
# baseline (speedup 1.0000x reference)
"""L1-distance sparse attention (nn_L1AttnSparse) on 8 Trainium2 cores.

Sharding: sequence-parallel over destination tokens. Core c owns dst rows
[c*512, (c+1)*512) for both batches and all 8 heads. The coo src indices are
resolved on the host into per-(dst,slot) gathered k/v row blocks laid out in
the exact SBUF tiling the device consumes (partition p = dst token of a
128-token chunk), so the device streams contiguous blocks at full HBM rate.
On-device: L1 distances, per-dst softmax over the 32 slots, and the weighted
v-sum on DVE/ACT, software-pipelined against the block loads.

kernel(**inputs) takes the full unsharded inputs and returns the full
[2, 4096, 8, 64] output.
"""

import numpy as np

import concourse.bacc as bacc
import concourse.bass as bass
import concourse.mybir as mybir
from concourse import tile
from concourse.bass_utils import run_bass_kernel_spmd

BS, N_TOK, NH, W = 2, 4096, 8, 64
DM = 32                      # slots per dst (dst_mxlen)
NCORE = 8
D_CORE = N_TOK // NCORE      # dst tokens per core (512)
P = 128                      # dst tokens per chunk = SBUF partitions
NCHUNK = D_CORE // P         # chunks per core (4)
NHG = 2                      # head groups
HPG = NH // NHG              # heads per group (4)
GW = HPG * W                 # row width per group (256 f32 = 1KB)
SCALE = -1.0 / float(np.sqrt(W))
MASK_BIG = 1e9               # added to z for absent (dst, slot) entries

dt = mybir.dt
AP = bass.AP
_CACHE: dict = {}


def _build_nc():
    nc = bacc.Bacc("TRN2", target_bir_lowering=False, debug=False)
    qd = nc.dram_tensor("qd", [BS, D_CORE, NH, W], dt.float32, kind="ExternalInput")
    kg_d = nc.dram_tensor("kg", [BS, NCHUNK, NHG, P, DM * GW], dt.float32,
                          kind="ExternalInput")
    vg_d = nc.dram_tensor("vg", [BS, NCHUNK, NHG, P, DM * GW], dt.float32,
                          kind="ExternalInput")
    mkz = nc.dram_tensor("mkz", [128, NCHUNK * DM], dt.float32, kind="ExternalInput")
    vo = nc.dram_tensor("vo", [BS, D_CORE, NH, W], dt.float32, kind="ExternalOutput")

    with tile.TileContext(nc) as tc:
        with (
            tc.tile_pool(name="const", bufs=1) as cpool,
            tc.tile_pool(name="io", bufs=2) as iopool,
            tc.tile_pool(name="work", bufs=4) as wpool,
            tc.tile_pool(name="stat", bufs=8) as spool,
            tc.tile_pool(name="outp", bufs=2) as opool,
        ):
            mk_sb = cpool.tile([128, NCHUNK * DM], dt.float32)
            nc.sync.dma_start(mk_sb[:], mkz[:])
            # q_sb[p, ((b*NCHUNK+ci)*NH+h)*W + d] = qd[b, ci*128+p, h, d]
            q_sb = cpool.tile([128, BS * NCHUNK * NH * W], dt.float32)
            q_src = AP(qd[:].tensor, 0, [
                [NH * W, P],                 # p (partition)
                [D_CORE * NH * W, BS],       # b
                [P * NH * W, NCHUNK],        # ci
                [1, NH * W],                 # (h, d)
            ])
            nc.sync.dma_start(q_sb[:], q_src)

            for b in range(BS):
                for ci in range(NCHUNK):
                    vo_acc = opool.tile([128, NH * W], dt.float32, tag="vo_acc")
                    mk_ap = mk_sb[:, ci * DM:(ci + 1) * DM]
                    for hg in range(NHG):
                        kg = iopool.tile([128, DM, GW], dt.float32, tag="kg")
                        vg = iopool.tile([128, DM, GW], dt.float32, tag="vg")
                        nc.sync.dma_start(kg[:], kg_d[b, ci, hg])
                        nc.sync.dma_start(vg[:], vg_d[b, ci, hg])
                        for h4 in range(HPG):
                            h = hg * HPG + h4
                            kh = kg[:, :, h4 * W:(h4 + 1) * W]
                            vh = vg[:, :, h4 * W:(h4 + 1) * W]
                            qoff = ((b * NCHUNK + ci) * NH + h) * W
                            q_ap = AP(q_sb[:].tensor, q_sb[:].offset + qoff,
                                      [q_sb[:].ap[0], [0, DM], [1, W]])

                            diff = wpool.tile([P, DM, W], dt.float32, tag="scr")
                            z = spool.tile([P, DM], dt.float32, tag="z")
                            zmin = spool.tile([P, 1], dt.float32, tag="zmin")
                            bias = spool.tile([P, 1], dt.float32, tag="bias")
                            e = spool.tile([P, DM], dt.float32, tag="e")
                            ssum = spool.tile([P, 1], dt.float32, tag="ssum")
                            rcp = spool.tile([P, 1], dt.float32, tag="rcp")
                            a = spool.tile([P, DM], dt.float32, tag="a")

                            # diff[p,s,d] = kg[p,s,d] - q[p,d]
                            nc.vector.tensor_tensor(out=diff[:], in0=kh, in1=q_ap,
                                                    op=mybir.AluOpType.subtract)
                            # z[p,s] = sum_d |diff|
                            nc.vector.tensor_reduce(out=z[:], in_=diff[:],
                                                    axis=mybir.AxisListType.X,
                                                    op=mybir.AluOpType.add,
                                                    apply_absolute_value=True)
                            # mask absent slots (z += 0 or 1e9)
                            nc.gpsimd.tensor_tensor(out=z[:], in0=z[:], in1=mk_ap,
                                                    op=mybir.AluOpType.add)
                            nc.vector.tensor_reduce(out=zmin[:], in_=z[:],
                                                    axis=mybir.AxisListType.X,
                                                    op=mybir.AluOpType.min)
                            # bias = -SCALE * zmin ; e = exp(SCALE*z + bias),
                            # ssum accumulated on ACT in the same op
                            nc.scalar.activation(out=bias[:], in_=zmin[:],
                                                 func=mybir.ActivationFunctionType.Copy,
                                                 scale=-SCALE)
                            nc.scalar.activation(out=e[:], in_=z[:],
                                                 func=mybir.ActivationFunctionType.Exp,
                                                 bias=bias[:], scale=SCALE,
                                                 accum_out=ssum[:])
                            nc.vector.reciprocal(out=rcp[:], in_=ssum[:])
                            # a = e * (1/ssum)  (ACT: Copy with per-partition scale)
                            nc.scalar.activation(out=a[:], in_=e[:],
                                                 func=mybir.ActivationFunctionType.Copy,
                                                 scale=rcp[:])
                            # mw[p, d*DM+s] = vg[p,s,d] * a[p,s]
                            # (on GPSIMD: DVE is the bottleneck engine, Pool
                            # is idle; reduces can't move off DVE but
                            # tensor_tensor can)
                            # reuse diff's storage for mw: diff is dead
                            # after the abs-reduce, and aliasing halves the
                            # scratch footprint so more heads pipeline
                            mw = diff
                            mw_t = AP(mw[:].tensor, mw[:].offset,
                                      [mw[:].ap[0], [1, DM], [DM, W]])
                            a_b = AP(a[:].tensor, a[:].offset,
                                     [a[:].ap[0], [1, DM], [0, W]])
                            nc.gpsimd.tensor_tensor(out=mw_t, in0=vh, in1=a_b,
                                                    op=mybir.AluOpType.mult)
                            # vo[p, h*W+d] = sum_s mw[p,d,s]
                            mw_v = AP(mw[:].tensor, mw[:].offset,
                                      [mw[:].ap[0], [DM, W], [1, DM]])
                            nc.vector.tensor_reduce(out=vo_acc[:, h * W:(h + 1) * W],
                                                    in_=mw_v,
                                                    axis=mybir.AxisListType.X,
                                                    op=mybir.AluOpType.add)
                    vo_dst = AP(vo[:].tensor,
                                (b * D_CORE + ci * P) * NH * W,
                                [[NH * W, P], [1, NH * W]])
                    nc.sync.dma_start(vo_dst, vo_acc[:])

    nc.compile()
    return nc


def _host_prep(v, q, k, coo):
    """Build per-core input maps from the full inputs (host-side shard+layout)."""
    v = np.ascontiguousarray(np.asarray(v, dtype=np.float32))
    q = np.ascontiguousarray(np.asarray(q, dtype=np.float32))
    k = np.ascontiguousarray(np.asarray(k, dtype=np.float32))
    coo = np.asarray(coo)

    src_tab = np.zeros((N_TOK, DM), np.int64)
    present = np.zeros((N_TOK, DM), bool)
    src_tab[coo[:, 0], coo[:, 2]] = coo[:, 1]
    present[coo[:, 0], coo[:, 2]] = True

    # [BS, N_TOK(dst), DM, NH, W] gathered rows
    kg_all = k[:, src_tab]
    vg_all = v[:, src_tab]

    in_maps = []
    for c in range(NCORE):
        r0 = c * D_CORE
        # [BS, D_CORE, DM, NH, W] -> [BS, NCHUNK, P, DM, NHG, GW]
        kg_c = kg_all[:, r0:r0 + D_CORE].reshape(
            BS, NCHUNK, P, DM, NHG, GW).transpose(0, 1, 4, 2, 3, 5)
        vg_c = vg_all[:, r0:r0 + D_CORE].reshape(
            BS, NCHUNK, P, DM, NHG, GW).transpose(0, 1, 4, 2, 3, 5)
        mkz_np = np.where(
            present[r0:r0 + D_CORE], 0.0, MASK_BIG).astype(np.float32)
        # [D_CORE, DM] -> [128, NCHUNK*DM]
        mkz_np = mkz_np.reshape(NCHUNK, P, DM).transpose(1, 0, 2).reshape(128, -1)
        in_maps.append({
            "qd": q[:, r0:r0 + D_CORE],
            "kg": np.ascontiguousarray(kg_c.reshape(BS, NCHUNK, NHG, P, DM * GW)),
            "vg": np.ascontiguousarray(vg_c.reshape(BS, NCHUNK, NHG, P, DM * GW)),
            "mkz": np.ascontiguousarray(mkz_np),
        })
    return in_maps


def _run(v, q, k, coo, trace=False, **spmd_kwargs):
    if "nc" not in _CACHE:
        _CACHE["nc"] = _build_nc()
    nc = _CACHE["nc"]
    in_maps = _host_prep(v, q, k, coo)
    res = run_bass_kernel_spmd(nc, in_maps, core_ids=list(range(NCORE)),
                               trace=trace, **spmd_kwargs)
    out = np.concatenate([r["vo"] for r in res.results], axis=1)
    return out.astype(np.float32), res


def kernel(v, q, k, coo, dst_mxlen=DM, **_ignored):
    assert int(dst_mxlen) == DM
    out, _ = _run(v, q, k, coo, trace=False)
    return out


# revision 7
# speedup vs baseline: 1.1173x; 1.1173x over previous
"""L1-distance sparse attention (nn_L1AttnSparse) on 8 Trainium2 cores.

Sharding: sequence-parallel over destination tokens. Core c owns dst rows
[c*512, (c+1)*512) for both batches and all 8 heads. The coo src indices are
resolved on the host into per-(dst,slot) gathered k/v row blocks laid out in
the exact SBUF tiling the device consumes (partition p = dst token of a
128-token chunk), so the device streams contiguous blocks at full HBM rate.
On-device: L1 distances, per-dst softmax over the 32 slots, and the weighted
v-sum on DVE/ACT, software-pipelined against the block loads.

kernel(**inputs) takes the full unsharded inputs and returns the full
[2, 4096, 8, 64] output.
"""

import numpy as np

import concourse.bacc as bacc
import concourse.bass as bass
import concourse.mybir as mybir
from concourse import tile
from concourse.bass_utils import run_bass_kernel_spmd

BS, N_TOK, NH, W = 2, 4096, 8, 64
DM = 32                      # slots per dst (dst_mxlen)
NCORE = 8
D_CORE = N_TOK // NCORE      # dst tokens per core (512)
P = 128                      # dst tokens per chunk = SBUF partitions
NCHUNK = D_CORE // P         # chunks per core (4)
NHG = 2                      # head groups
HPG = NH // NHG              # heads per group (4)
GW = HPG * W                 # row width per group (256 f32 = 1KB)
SCALE = -1.0 / float(np.sqrt(W))
MASK_BIG = 1e9               # added to z for absent (dst, slot) entries

dt = mybir.dt
AP = bass.AP
_CACHE: dict = {}


def _build_nc():
    nc = bacc.Bacc("TRN2", target_bir_lowering=False, debug=False)
    qd = nc.dram_tensor("qd", [BS, D_CORE, NH, W], dt.float32, kind="ExternalInput")
    kg_d = nc.dram_tensor("kg", [BS, NCHUNK, NHG, P, DM * GW], dt.float32,
                          kind="ExternalInput")
    vg_d = nc.dram_tensor("vg", [BS, NCHUNK, NHG, P, DM * GW], dt.float32,
                          kind="ExternalInput")
    mkz = nc.dram_tensor("mkz", [128, NCHUNK * DM], dt.float32, kind="ExternalInput")
    vo = nc.dram_tensor("vo", [BS, D_CORE, NH, W], dt.float32, kind="ExternalOutput")

    with tile.TileContext(nc) as tc:
        with (
            tc.tile_pool(name="const", bufs=1) as cpool,
            tc.tile_pool(name="io", bufs=2) as iopool,
            tc.tile_pool(name="work", bufs=4) as wpool,
            tc.tile_pool(name="stat", bufs=8) as spool,
            tc.tile_pool(name="outp", bufs=2) as opool,
        ):
            mk_sb = cpool.tile([128, NCHUNK * DM], dt.float32)
            nc.sync.dma_start(mk_sb[:], mkz[:])
            # q_sb[p, ((b*NCHUNK+ci)*NH+h)*W + d] = qd[b, ci*128+p, h, d]
            q_sb = cpool.tile([128, BS * NCHUNK * NH * W], dt.float32)
            q_src = AP(qd[:].tensor, 0, [
                [NH * W, P],                 # p (partition)
                [D_CORE * NH * W, BS],       # b
                [P * NH * W, NCHUNK],        # ci
                [1, NH * W],                 # (h, d)
            ])
            nc.sync.dma_start(q_sb[:], q_src)

            for b in range(BS):
                for ci in range(NCHUNK):
                    vo_acc = opool.tile([128, NH * W], dt.float32, tag="vo_acc")
                    mk_ap = mk_sb[:, ci * DM:(ci + 1) * DM]
                    for hg in range(NHG):
                        kg = iopool.tile([128, DM, GW], dt.float32, tag="kg")
                        vg = iopool.tile([128, DM, GW], dt.float32, tag="vg")
                        nc.sync.dma_start(kg[:], kg_d[b, ci, hg])
                        nc.sync.dma_start(vg[:], vg_d[b, ci, hg])
                        for h4 in range(HPG):
                            h = hg * HPG + h4
                            kh = kg[:, :, h4 * W:(h4 + 1) * W]
                            vh = vg[:, :, h4 * W:(h4 + 1) * W]
                            qoff = ((b * NCHUNK + ci) * NH + h) * W
                            q_ap = AP(q_sb[:].tensor, q_sb[:].offset + qoff,
                                      [q_sb[:].ap[0], [0, DM], [1, W]])

                            diff = wpool.tile([P, DM, W], dt.float32, tag="scr")
                            z = spool.tile([P, DM], dt.float32, tag="z")
                            e = spool.tile([P, DM], dt.float32, tag="e")
                            ssum = spool.tile([P, 1], dt.float32, tag="ssum")
                            rcp = spool.tile([P, 1], dt.float32, tag="rcp")
                            a = spool.tile([P, DM], dt.float32, tag="a")

                            # diff[p,s,d] = kg[p,s,d] - q[p,d]
                            nc.vector.tensor_tensor(out=diff[:], in0=kh, in1=q_ap,
                                                    op=mybir.AluOpType.subtract)
                            # z[p,s] = sum_d |diff|
                            nc.vector.tensor_reduce(out=z[:], in_=diff[:],
                                                    axis=mybir.AxisListType.X,
                                                    op=mybir.AluOpType.add,
                                                    apply_absolute_value=True)
                            # mask absent slots (z += 0 or 1e9)
                            nc.gpsimd.tensor_tensor(out=z[:], in0=z[:], in1=mk_ap,
                                                    op=mybir.AluOpType.add)
                            # e = exp(SCALE*z), ssum accumulated on ACT in
                            # the same op. No max-subtraction needed: z >= 0
                            # so SCALE*z <= 0 (no overflow), and f32 exp
                            # underflow needs z > ~2800 while z <= 64*|q-k|
                            # stays orders of magnitude below; masked slots
                            # (z += 1e9) flush to exactly 0.
                            nc.scalar.activation(out=e[:], in_=z[:],
                                                 func=mybir.ActivationFunctionType.Exp,
                                                 scale=SCALE,
                                                 accum_out=ssum[:])
                            nc.vector.reciprocal(out=rcp[:], in_=ssum[:])
                            # a = e * (1/ssum)  (ACT: Copy with per-partition scale)
                            nc.scalar.activation(out=a[:], in_=e[:],
                                                 func=mybir.ActivationFunctionType.Copy,
                                                 scale=rcp[:])
                            # mw[p, d*DM+s] = vg[p,s,d] * a[p,s]
                            # (on GPSIMD: DVE is the bottleneck engine, Pool
                            # is idle; reduces can't move off DVE but
                            # tensor_tensor can)
                            # reuse diff's storage for mw: diff is dead
                            # after the abs-reduce, and aliasing halves the
                            # scratch footprint so more heads pipeline
                            mw = diff
                            mw_t = AP(mw[:].tensor, mw[:].offset,
                                      [mw[:].ap[0], [1, DM], [DM, W]])
                            a_b = AP(a[:].tensor, a[:].offset,
                                     [a[:].ap[0], [1, DM], [0, W]])
                            nc.gpsimd.tensor_tensor(out=mw_t, in0=vh, in1=a_b,
                                                    op=mybir.AluOpType.mult)
                            # vo[p, h*W+d] = sum_s mw[p,d,s]
                            mw_v = AP(mw[:].tensor, mw[:].offset,
                                      [mw[:].ap[0], [DM, W], [1, DM]])
                            nc.vector.tensor_reduce(out=vo_acc[:, h * W:(h + 1) * W],
                                                    in_=mw_v,
                                                    axis=mybir.AxisListType.X,
                                                    op=mybir.AluOpType.add)
                    vo_dst = AP(vo[:].tensor,
                                (b * D_CORE + ci * P) * NH * W,
                                [[NH * W, P], [1, NH * W]])
                    nc.sync.dma_start(vo_dst, vo_acc[:])

    nc.compile()
    return nc


def _host_prep(v, q, k, coo):
    """Build per-core input maps from the full inputs (host-side shard+layout)."""
    v = np.ascontiguousarray(np.asarray(v, dtype=np.float32))
    q = np.ascontiguousarray(np.asarray(q, dtype=np.float32))
    k = np.ascontiguousarray(np.asarray(k, dtype=np.float32))
    coo = np.asarray(coo)

    src_tab = np.zeros((N_TOK, DM), np.int64)
    present = np.zeros((N_TOK, DM), bool)
    src_tab[coo[:, 0], coo[:, 2]] = coo[:, 1]
    present[coo[:, 0], coo[:, 2]] = True

    # [BS, N_TOK(dst), DM, NH, W] gathered rows
    kg_all = k[:, src_tab]
    vg_all = v[:, src_tab]

    in_maps = []
    for c in range(NCORE):
        r0 = c * D_CORE
        # [BS, D_CORE, DM, NH, W] -> [BS, NCHUNK, P, DM, NHG, GW]
        kg_c = kg_all[:, r0:r0 + D_CORE].reshape(
            BS, NCHUNK, P, DM, NHG, GW).transpose(0, 1, 4, 2, 3, 5)
        vg_c = vg_all[:, r0:r0 + D_CORE].reshape(
            BS, NCHUNK, P, DM, NHG, GW).transpose(0, 1, 4, 2, 3, 5)
        mkz_np = np.where(
            present[r0:r0 + D_CORE], 0.0, MASK_BIG).astype(np.float32)
        # [D_CORE, DM] -> [128, NCHUNK*DM]
        mkz_np = mkz_np.reshape(NCHUNK, P, DM).transpose(1, 0, 2).reshape(128, -1)
        in_maps.append({
            "qd": q[:, r0:r0 + D_CORE],
            "kg": np.ascontiguousarray(kg_c.reshape(BS, NCHUNK, NHG, P, DM * GW)),
            "vg": np.ascontiguousarray(vg_c.reshape(BS, NCHUNK, NHG, P, DM * GW)),
            "mkz": np.ascontiguousarray(mkz_np),
        })
    return in_maps


def _run(v, q, k, coo, trace=False, **spmd_kwargs):
    if "nc" not in _CACHE:
        _CACHE["nc"] = _build_nc()
    nc = _CACHE["nc"]
    in_maps = _host_prep(v, q, k, coo)
    res = run_bass_kernel_spmd(nc, in_maps, core_ids=list(range(NCORE)),
                               trace=trace, **spmd_kwargs)
    out = np.concatenate([r["vo"] for r in res.results], axis=1)
    return out.astype(np.float32), res


def kernel(v, q, k, coo, dst_mxlen=DM, **_ignored):
    assert int(dst_mxlen) == DM
    out, _ = _run(v, q, k, coo, trace=False)
    return out


# revision 8
# speedup vs baseline: 1.1221x; 1.0044x over previous
"""L1-distance sparse attention (nn_L1AttnSparse) on 8 Trainium2 cores.

Sharding: sequence-parallel over destination tokens. Core c owns dst rows
[c*512, (c+1)*512) for both batches and all 8 heads. The coo src indices are
resolved on the host into per-(dst,slot) gathered k/v row blocks laid out in
the exact SBUF tiling the device consumes (partition p = dst token of a
128-token chunk), so the device streams contiguous blocks at full HBM rate.
On-device: L1 distances, per-dst softmax over the 32 slots, and the weighted
v-sum on DVE/ACT, software-pipelined against the block loads.

kernel(**inputs) takes the full unsharded inputs and returns the full
[2, 4096, 8, 64] output.
"""

import numpy as np

import concourse.bacc as bacc
import concourse.bass as bass
import concourse.mybir as mybir
from concourse import tile
from concourse.bass_utils import run_bass_kernel_spmd

BS, N_TOK, NH, W = 2, 4096, 8, 64
DM = 32                      # slots per dst (dst_mxlen)
NCORE = 8
D_CORE = N_TOK // NCORE      # dst tokens per core (512)
P = 128                      # dst tokens per chunk = SBUF partitions
NCHUNK = D_CORE // P         # chunks per core (4)
NHG = 2                      # head groups
HPG = NH // NHG              # heads per group (4)
GW = HPG * W                 # row width per group (256 f32 = 1KB)
SCALE = -1.0 / float(np.sqrt(W))
MASK_BIG = 1e9               # added to z for absent (dst, slot) entries

dt = mybir.dt
AP = bass.AP
_CACHE: dict = {}


def _build_nc():
    nc = bacc.Bacc("TRN2", target_bir_lowering=False, debug=False)
    qd = nc.dram_tensor("qd", [BS, D_CORE, NH, W], dt.float32, kind="ExternalInput")
    kg_d = nc.dram_tensor("kg", [BS, NCHUNK, NHG, P, DM * GW], dt.float32,
                          kind="ExternalInput")
    vg_d = nc.dram_tensor("vg", [BS, NCHUNK, NHG, P, DM * GW], dt.float32,
                          kind="ExternalInput")
    vo = nc.dram_tensor("vo", [BS, D_CORE, NH, W], dt.float32, kind="ExternalOutput")

    with tile.TileContext(nc) as tc:
        with (
            tc.tile_pool(name="const", bufs=1) as cpool,
            tc.tile_pool(name="io", bufs=2) as iopool,
            tc.tile_pool(name="work", bufs=4) as wpool,
            tc.tile_pool(name="stat", bufs=8) as spool,
            tc.tile_pool(name="outp", bufs=2) as opool,
        ):
            # q_sb[p, ((b*NCHUNK+ci)*NH+h)*W + d] = qd[b, ci*128+p, h, d]
            q_sb = cpool.tile([128, BS * NCHUNK * NH * W], dt.float32)
            q_src = AP(qd[:].tensor, 0, [
                [NH * W, P],                 # p (partition)
                [D_CORE * NH * W, BS],       # b
                [P * NH * W, NCHUNK],        # ci
                [1, NH * W],                 # (h, d)
            ])
            nc.sync.dma_start(q_sb[:], q_src)

            for b in range(BS):
                for ci in range(NCHUNK):
                    vo_acc = opool.tile([128, NH * W], dt.float32, tag="vo_acc")
                    for hg in range(NHG):
                        kg = iopool.tile([128, DM, GW], dt.float32, tag="kg")
                        vg = iopool.tile([128, DM, GW], dt.float32, tag="vg")
                        nc.sync.dma_start(kg[:], kg_d[b, ci, hg])
                        nc.sync.dma_start(vg[:], vg_d[b, ci, hg])
                        for h4 in range(HPG):
                            h = hg * HPG + h4
                            kh = kg[:, :, h4 * W:(h4 + 1) * W]
                            vh = vg[:, :, h4 * W:(h4 + 1) * W]
                            qoff = ((b * NCHUNK + ci) * NH + h) * W
                            q_ap = AP(q_sb[:].tensor, q_sb[:].offset + qoff,
                                      [q_sb[:].ap[0], [0, DM], [1, W]])

                            diff = wpool.tile([P, DM, W], dt.float32, tag="scr")
                            z = spool.tile([P, DM], dt.float32, tag="z")
                            e = spool.tile([P, DM], dt.float32, tag="e")
                            ssum = spool.tile([P, 1], dt.float32, tag="ssum")
                            rcp = spool.tile([P, 1], dt.float32, tag="rcp")
                            a = spool.tile([P, DM], dt.float32, tag="a")

                            # diff[p,s,d] = kg[p,s,d] - q[p,d]
                            nc.vector.tensor_tensor(out=diff[:], in0=kh, in1=q_ap,
                                                    op=mybir.AluOpType.subtract)
                            # z[p,s] = sum_d |diff|
                            nc.vector.tensor_reduce(out=z[:], in_=diff[:],
                                                    axis=mybir.AxisListType.X,
                                                    op=mybir.AluOpType.add,
                                                    apply_absolute_value=True)
                            # e = exp(SCALE*z), ssum accumulated on ACT in
                            # the same op. No max-subtraction needed: z >= 0
                            # so SCALE*z <= 0 (no overflow), and f32 exp
                            # underflow needs z > ~2800 while z <= 64*|q-k|
                            # stays orders of magnitude below; masked slots
                            # (z += 1e9) flush to exactly 0.
                            nc.scalar.activation(out=e[:], in_=z[:],
                                                 func=mybir.ActivationFunctionType.Exp,
                                                 scale=SCALE,
                                                 accum_out=ssum[:])
                            nc.vector.reciprocal(out=rcp[:], in_=ssum[:])
                            # a = e * (1/ssum)  (ACT: Copy with per-partition scale)
                            nc.scalar.activation(out=a[:], in_=e[:],
                                                 func=mybir.ActivationFunctionType.Copy,
                                                 scale=rcp[:])
                            # mw[p, d*DM+s] = vg[p,s,d] * a[p,s]
                            # (on GPSIMD: DVE is the bottleneck engine, Pool
                            # is idle; reduces can't move off DVE but
                            # tensor_tensor can)
                            # reuse diff's storage for mw: diff is dead
                            # after the abs-reduce, and aliasing halves the
                            # scratch footprint so more heads pipeline
                            mw = diff
                            mw_t = AP(mw[:].tensor, mw[:].offset,
                                      [mw[:].ap[0], [1, DM], [DM, W]])
                            a_b = AP(a[:].tensor, a[:].offset,
                                     [a[:].ap[0], [1, DM], [0, W]])
                            nc.gpsimd.tensor_tensor(out=mw_t, in0=vh, in1=a_b,
                                                    op=mybir.AluOpType.mult)
                            # vo[p, h*W+d] = sum_s mw[p,d,s]
                            mw_v = AP(mw[:].tensor, mw[:].offset,
                                      [mw[:].ap[0], [DM, W], [1, DM]])
                            nc.vector.tensor_reduce(out=vo_acc[:, h * W:(h + 1) * W],
                                                    in_=mw_v,
                                                    axis=mybir.AxisListType.X,
                                                    op=mybir.AluOpType.add)
                    vo_dst = AP(vo[:].tensor,
                                (b * D_CORE + ci * P) * NH * W,
                                [[NH * W, P], [1, NH * W]])
                    nc.sync.dma_start(vo_dst, vo_acc[:])

    nc.compile()
    return nc


def _host_prep(v, q, k, coo):
    """Build per-core input maps from the full inputs (host-side shard+layout)."""
    v = np.ascontiguousarray(np.asarray(v, dtype=np.float32))
    q = np.ascontiguousarray(np.asarray(q, dtype=np.float32))
    k = np.ascontiguousarray(np.asarray(k, dtype=np.float32))
    coo = np.asarray(coo)

    src_tab = np.zeros((N_TOK, DM), np.int64)
    present = np.zeros((N_TOK, DM), bool)
    src_tab[coo[:, 0], coo[:, 2]] = coo[:, 1]
    present[coo[:, 0], coo[:, 2]] = True

    # [BS, N_TOK(dst), DM, NH, W] gathered rows. Absent (dst,slot) entries
    # are masked in-data: k-row = 1e9 makes z ~ 6e10 so exp(SCALE*z) == 0
    # exactly, and the zeroed v-row contributes nothing to the numerator.
    kg_all = k[:, src_tab]
    vg_all = v[:, src_tab]
    absent = ~present
    if absent.any():
        kg_all[:, absent] = MASK_BIG
        vg_all[:, absent] = 0.0

    in_maps = []
    for c in range(NCORE):
        r0 = c * D_CORE
        # [BS, D_CORE, DM, NH, W] -> [BS, NCHUNK, P, DM, NHG, GW]
        kg_c = kg_all[:, r0:r0 + D_CORE].reshape(
            BS, NCHUNK, P, DM, NHG, GW).transpose(0, 1, 4, 2, 3, 5)
        vg_c = vg_all[:, r0:r0 + D_CORE].reshape(
            BS, NCHUNK, P, DM, NHG, GW).transpose(0, 1, 4, 2, 3, 5)
        in_maps.append({
            "qd": q[:, r0:r0 + D_CORE],
            "kg": np.ascontiguousarray(kg_c.reshape(BS, NCHUNK, NHG, P, DM * GW)),
            "vg": np.ascontiguousarray(vg_c.reshape(BS, NCHUNK, NHG, P, DM * GW)),
        })
    return in_maps


def _run(v, q, k, coo, trace=False, **spmd_kwargs):
    if "nc" not in _CACHE:
        _CACHE["nc"] = _build_nc()
    nc = _CACHE["nc"]
    in_maps = _host_prep(v, q, k, coo)
    res = run_bass_kernel_spmd(nc, in_maps, core_ids=list(range(NCORE)),
                               trace=trace, **spmd_kwargs)
    out = np.concatenate([r["vo"] for r in res.results], axis=1)
    return out.astype(np.float32), res


def kernel(v, q, k, coo, dst_mxlen=DM, **_ignored):
    assert int(dst_mxlen) == DM
    out, _ = _run(v, q, k, coo, trace=False)
    return out
